# revision 61
# baseline (speedup 1.0000x reference)
"""Trainium2 Bass kernel for nn_AttentionModule (SAGAN-style 2D self-attention).

Per-sample computation (B=8 samples, one per NeuronCore, data-parallel):
    q = Wq @ x + bq         (32, 4096)
    k = Wk @ x + bk         (32, 4096)
    v = Wv @ x              (256, 4096)   [bv added on host: sum(softmax)=1]
    attn = softmax(q^T k)   (4096, 4096), softmax over last dim
    y = v @ attn^T + x      (256, 4096)

Strategy per core:
  - Scores computed TRANSPOSED: Lt[m, n] = sum_d k[d,m] q[d,n], so the
    exp'd scores P land in SBUF with the contraction dim (m) on partitions,
    directly usable as the stationary operand of the AV matmul.
  - No softmax max-subtraction (|logits| < ~25, exp fits fp32 comfortably).
  - Softmax denominator comes free: the AV streaming operand v' carries an
    extra ones-column (memset once), so outT[n, 256] = colsum(P).
  - v-bias bv is NOT applied on device: softmax weights sum to 1, so bv
    passes through the attention average and is added on the host.
  - Logits matmuls (K = D = 32) are 3x row-packed via tile_position with
    the row band rotating across groups ((3g+r)%4), so the next group's
    LDWEIGHTS pull ahead of in-flight matmuls; packs are emitted in
    back-to-back pairs, and the last 2 packs of each chunk are held back
    to fill the PE bubble during the j==3 finalize (DVE recip/normalize).
  - AV in bf16 (P needs range up to ~5e10: bf16, not fp16); fp8 is
    impossible (needs a per-column max subtraction = partition-dim
    reduction). Logits/projections/transposes fp16 (f32r matmuls 2-4x
    slow; matmul PSUM out must be fp32).
  - Softmax-exp on ACT, [128, 1536] per 3-bank PSUM group, double
    buffered. ACT is the co-bottleneck at ~145us busy vs PE ~175us.
  - Final: per n-block normalize by 1/colsum, PE-transpose back to [c, n],
    add fp16 residual, DMA out fp16 (host upcasts + adds bv).
  - Host-side prep packs every SBUF-resident tensor so it loads in ONE
    DMA (x interleaved [128, 2, HW]; weights pre-tiled/4x-replicated).
  - x DMAs all ride ONE ring (sync) so they transfer serially at full HBM
    bandwidth, part p landing just before projection p needs it; first
    two parts split in half so compute starts ~1us earlier. v'-proj
    evacuates 2 m-blocks per 1-bank PSUM tile to halve rotation stalls.
"""

import numpy as np

import concourse.bacc as bacc
import concourse.bass as bass
import concourse.mybir as mybir
import concourse.tile as tile

B, C, D = 8, 256, 32
HW = 4096                      # 64*64 pixels
NCH = 8                        # n-chunks of 512
CHUNK = 512
NB = 128                       # n-block
MB = 128                       # m-block
NMB = HW // MB                 # 32 m-blocks
VW = 258                       # v' row width: 256 c + colsum-ones + pad
GS = [2] + [3] * 10            # m-blocks per logits group (sum = 32);
                               # small group first so the 2 reserved
                               # flush packs are both 3-wide
GOFF = [0]
for _g in GS:
    GOFF.append(GOFF[-1] + _g)  # group -> first m-block
NG = len(GS)                   # 11 logits groups per chunk
F32 = mybir.dt.float32
BF16 = mybir.dt.bfloat16
FP16 = mybir.dt.float16
AF = mybir.ActivationFunctionType


def build_nc():
    nc = bacc.Bacc("TRN2", target_bir_lowering=False, debug=False)
    t = {}
    t["x"] = nc.dram_tensor("x", [128, 2, HW], FP16, kind="ExternalInput").ap()
    t["wq4"] = nc.dram_tensor("wq4", [128, 2, 128], FP16,
                              kind="ExternalInput").ap()
    t["wk4"] = nc.dram_tensor("wk4", [128, 2, 128], FP16,
                              kind="ExternalInput").ap()
    t["bqk"] = nc.dram_tensor("bqk", [128, 2], F32, kind="ExternalInput").ap()
    t["wvt"] = nc.dram_tensor("wvt", [128, 2, 256], FP16,
                              kind="ExternalInput").ap()
    t["xt"] = nc.dram_tensor("xt", [128, NMB, 256], FP16,
                             kind="ExternalInput").ap()
    t["y"] = nc.dram_tensor("y", [128, NMB, 256], FP16,
                            kind="ExternalOutput").ap()

    with tile.TileContext(nc) as tc:
        _emit(nc, tc, t)
    nc.compile()
    return nc


def _emit(nc, tc, t):
    with (
        tc.tile_pool(name="const", bufs=1) as const,
        tc.tile_pool(name="sb", bufs=1) as sb,
        tc.tile_pool(name="stage", bufs=2) as stage,
    ):
        # ---- constants / weights (one DMA per SBUF tile) -------------
        wq4 = const.tile([128, 2, 128], FP16)  # [c', cc, 4x32 q-weights]
        wk4 = const.tile([128, 2, 128], FP16)
        wvt = const.tile([128, 2, 256], FP16)  # [c', cc, 256 v-weights]
        bqk = const.tile([128, 2], F32)        # col 0: bq4, col 1: bk4
        nc.gpsimd.dma_start(wk4, t["wk4"])
        nc.gpsimd.dma_start(wq4, t["wq4"])
        nc.gpsimd.dma_start(bqk, t["bqk"])

        # ---- persistent SBUF tensors ---------------------------------
        x16 = sb.tile([128, 2, HW], FP16)      # x (fp16): projections
        xt = sb.tile([128, NMB, 256], FP16)    # x transposed [n, c]: residual
        q4 = sb.tile([128, HW], FP16)          # q replicated 4x on partitions
        k4 = sb.tile([128, HW], FP16)
        vp = sb.tile([128, NMB, VW], BF16)     # v' tiles: [m-chunk 128, VW]
        pbuf = [sb.tile([128, 16 * 1024], BF16, tag=f"p{i}", name=f"p{i}")
                for i in range(2)]

        # ones column for the free colsum; pad col zeroed
        nc.gpsimd.memset(vp[:, :, 256:257], 1.0)
        nc.gpsimd.memset(vp[:, :, 257:258], 0.0)

        # x: 8 DMAs of [128, 2, 512], ALL on the sync queue: same-ring DMAs
        # execute serially at full HBM bandwidth, so part p lands at
        # ~8.6+0.9p us, matching the projection cadence (concurrent rings
        # would share bandwidth and deliver part 0 only after ~12.6us)
        xsplits = [0, 256, 512, 768, 1024] + [CHUNK * p for p in range(3, 9)]
        for a, b in zip(xsplits, xsplits[1:]):
            nc.sync.dma_start(x16[:, :, a:b], t["x"][:, :, a:b])
        nc.gpsimd.dma_start(wvt, t["wvt"])
        nc.gpsimd.dma_start(xt, t["xt"])  # residual copy, needed from ~45us

        # ---- main loop -----------------------------------------------
        # PSUM: lt 2-bank x3 bufs = 6 banks; "avtr" shared tag (av accum /
        # transpose out / v'-proj / qk-proj) 1 bank x2 bufs = 2. Total 8.
        with tc.tile_pool(name="ps1", bufs=1, space="PSUM") as ps1:

            def qk_proj(w4, bcol, dst, ch, tag):
                s = slice(CHUNK * ch, CHUNK * (ch + 1))
                pt = ps1.tile([128, CHUNK], F32, tag=tag, name="pt", bufs=2)
                for cc in range(2):
                    nc.tensor.matmul(
                        pt, w4[:, cc, :], x16[:, cc, s],
                        start=(cc == 0), stop=(cc == 1),
                    )
                nc.vector.tensor_scalar_add(dst[:, s], pt, bcol)

            def logits_group(ch, g):
                """GS[g] row-packed matmuls (m-blocks GOFF[g]..) + exp.

                Row bands rotate across groups so the next group's
                LDWEIGHTS can pull ahead of in-flight matmuls."""
                sz = GS[g]
                lt = ps1.tile([128, 1536], F32, tag="lt", bufs=2, name="lt")
                ns = slice(CHUNK * ch, CHUNK * (ch + 1))
                for r in range(sz):
                    mb = GOFF[g] + r
                    b = (3 * g + r) % 4
                    nc.tensor.matmul(
                        lt[:, CHUNK * r:CHUNK * (r + 1)],
                        k4[32 * b:32 * (b + 1), MB * mb:MB * (mb + 1)],
                        q4[32 * b:32 * (b + 1), ns],
                        start=True, stop=True, tile_position=(32 * b, 0),
                    )
                dst = pbuf[ch % 2][:, CHUNK * GOFF[g]:CHUNK * GOFF[g + 1]]
                nc.scalar.activation(dst, lt[:, 0:CHUNK * sz], AF.Exp)

            def vt_unit(mb2):
                """v' tiles 2*mb2, 2*mb2+1: 4 matmuls into one 1-bank PSUM
                tile + a single copy (halves the PSUM-rotation stalls)."""
                vt = ps1.tile([128, 2, 256], F32, tag="avtr", bufs=2,
                              name="vt")
                for b in range(2):
                    mb = 2 * mb2 + b
                    ms = slice(MB * mb, MB * (mb + 1))
                    for cc in range(2):
                        nc.tensor.matmul(
                            vt[:, b, :], x16[:, cc, ms], wvt[:, cc, :],
                            start=(cc == 0), stop=(cc == 1),
                        )
                if mb2 % 4 == 0:
                    nc.scalar.activation(vp[:, 2 * mb2:2 * mb2 + 2, 0:256],
                                         vt, AF.Identity)
                else:
                    nc.vector.tensor_copy(vp[:, 2 * mb2:2 * mb2 + 2, 0:256],
                                          vt)

            def av_unit(ch, j, mc):
                off = CHUNK * mc  # m-blocks pack contiguously in pbuf
                nc.tensor.matmul(
                    t["avps"], pbuf[ch % 2][:, off + NB * j:off + NB * (j + 1)],
                    vp[:, mc, :],
                    start=(mc == 0), stop=(mc == 31),
                )

            def finalize(ch, j, ysb):
                """All-DVE: normalize by 1/colsum, add the [n, c]-layout
                residual; output stays [n, c] (host transposes back)."""
                avps = t["avps"]
                recip = stage.tile([128, 1], F32, tag="recip", name="recip")
                nc.vector.reciprocal(recip, avps[:, 256:257])
                normt = stage.tile([128, 256], FP16, tag="normt", name="normt")
                nc.vector.tensor_scalar_mul(normt, avps[:, 0:256], recip)
                nc.vector.tensor_tensor(
                    out=ysb[:, j, :], in0=normt, in1=xt[:, 4 * ch + j, :],
                    op=mybir.AluOpType.add,
                )

            # k4 chunks required before logits pack g can run (cols 384g..)
            KREQ = [min((128 * GOFF[g + 1] - 1) // CHUNK, NCH - 1)
                    for g in range(NG)]
            for ch in range(NCH + 1):
                # filler units for this pipeline stage:
                #  ch == 0  -> k4/q4 proj (u<8), 16 paired v'-proj units
                #  ch >= 1  -> 128 AV matmuls of chunk ch-1 (+finalize/4)
                n_units = (NCH + NMB // 2) if ch == 0 else 128
                ysb = None
                if ch > 0:
                    ysb = stage.tile([128, 4, 256], FP16, tag="y", name="ysb")
                g_next = 0
                # packs are emitted in back-to-back pairs so the 2nd pack's
                # LDWEIGHTS hide under the 1st pack's MMs
                for u in range(n_units):
                    if ch < NCH:
                        while (g_next < NG
                               and g_next <= (u * NG) // n_units
                               and not (ch == 0 and KREQ[g_next] >= u)):
                            logits_group(ch, g_next)
                            g_next += 1
                            if ch >= 1 and g_next < NG:
                                logits_group(ch, g_next)
                                g_next += 1
                    if ch == 0:
                        if u < NCH:
                            qk_proj(wk4, bqk[:, 1:2], k4, u, "avtr")
                            qk_proj(wq4, bqk[:, 0:1], q4, u, "avtr")
                        else:
                            vt_unit(u - NCH)
                    else:
                        j, mc = divmod(u, 32)
                        if mc == 0:
                            t["avps"] = ps1.tile([128, VW], F32, tag="avtr",
                                                 bufs=2, name="avps")
                        av_unit(ch - 1, j, mc)
                        if mc == 31:
                            finalize(ch - 1, j, ysb)
                if ch < NCH:
                    while g_next < NG:
                        logits_group(ch, g_next)
                        g_next += 1
                if ch > 0:
                    nc.sync.dma_start(t["y"][:, 4 * (ch - 1):4 * ch, :], ysb)


# ---------------------------------------------------------------------
# host-side wrapper
# ---------------------------------------------------------------------
_CACHE = {}


def _prep_shared(Wq, bq, Wk, bk, Wv, bv):
    wq4 = np.tile(Wq.T, (1, 4)).reshape(2, 128, 128).transpose(1, 0, 2)
    wk4 = np.tile(Wk.T, (1, 4)).reshape(2, 128, 128).transpose(1, 0, 2)
    bqk = np.stack([np.tile(bq, 4), np.tile(bk, 4)], axis=1)
    wvt = Wv.T.reshape(2, 128, 256).transpose(1, 0, 2)
    return {"wq4": np.ascontiguousarray(wq4, dtype=np.float16),
            "wk4": np.ascontiguousarray(wk4, dtype=np.float16),
            "bqk": np.ascontiguousarray(bqk, dtype=np.float32),
            "wvt": np.ascontiguousarray(wvt, dtype=np.float16)}


def make_in_maps(x, Wq, bq, Wk, bk, Wv, bv):
    x = np.asarray(x, dtype=np.float32).reshape(B, C, HW)
    xi = x.reshape(B, 2, 128, HW).transpose(0, 2, 1, 3).astype(np.float16)
    # transposed residual copy: xt[p, nb, c] = x[c, 128*nb + p]
    xts = x.reshape(B, C, NMB, 128).transpose(0, 3, 2, 1).astype(np.float16)
    shared = _prep_shared(*(np.asarray(a, dtype=np.float32)
                            for a in (Wq, bq, Wk, bk, Wv, bv)))
    return [{"x": np.ascontiguousarray(xi[b]),
             "xt": np.ascontiguousarray(xts[b]), **shared} for b in range(B)]


def postprocess(res, bv):
    yt = np.stack([res.results[b]["y"] for b in range(B)])  # [B,128,NMB,256]
    y = yt.astype(np.float32).transpose(0, 3, 2, 1)  # [B, c, nb, p]
    y = np.ascontiguousarray(y).reshape(B, C, HW)
    y += np.asarray(bv, dtype=np.float32)[None, :, None]
    return y.reshape(B, C, 64, 64)


def kernel(x, Wq, bq, Wk, bk, Wv, bv):
    from concourse.bass_utils import run_bass_kernel_spmd

    in_maps = make_in_maps(x, Wq, bq, Wk, bk, Wv, bv)
    if "nc" not in _CACHE:
        _CACHE["nc"] = build_nc()
    res = run_bass_kernel_spmd(_CACHE["nc"], in_maps, core_ids=list(range(B)))
    return postprocess(res, bv)


# revision 62
# speedup vs baseline: 1.0768x; 1.0768x over previous
"""Trainium2 Bass kernel for nn_AttentionModule (SAGAN-style 2D self-attention).

Per-sample computation (B=8 samples, one per NeuronCore, data-parallel):
    q = Wq @ x + bq         (32, 4096)
    k = Wk @ x + bk         (32, 4096)
    v = Wv @ x              (256, 4096)   [bv added on host: sum(softmax)=1]
    attn = softmax(q^T k)   (4096, 4096), softmax over last dim
    y = v @ attn^T + x      (256, 4096)

Strategy per core:
  - Scores computed TRANSPOSED: Lt[m, n] = sum_d k[d,m] q[d,n], so the
    exp'd scores P land in SBUF with the contraction dim (m) on partitions,
    directly usable as the stationary operand of the AV matmul.
  - No softmax max-subtraction (|logits| < ~25, exp fits fp32 comfortably).
  - Softmax denominator comes free: the AV streaming operand v' carries an
    extra ones-column (memset once), so outT[n, 256] = colsum(P).
  - v-bias bv is NOT applied on device: softmax weights sum to 1, so bv
    passes through the attention average and is added on the host.
  - Logits matmuls (K = D = 32) are 3x row-packed via tile_position with
    the row band rotating across groups ((3g+r)%4), so the next group's
    LDWEIGHTS pull ahead of in-flight matmuls; packs are emitted in
    back-to-back pairs, and the last 2 packs of each chunk are held back
    to fill the PE bubble during the j==3 finalize (DVE recip/normalize).
  - AV in bf16 (P needs range up to ~5e10: bf16, not fp16); fp8 is
    impossible (needs a per-column max subtraction = partition-dim
    reduction). Logits/projections/transposes fp16 (f32r matmuls 2-4x
    slow; matmul PSUM out must be fp32).
  - Softmax-exp on ACT, [128, 1536] per 3-bank PSUM group, double
    buffered. ACT is the co-bottleneck at ~145us busy vs PE ~175us.
  - Final: per n-block normalize by 1/colsum, PE-transpose back to [c, n],
    add fp16 residual, DMA out fp16 (host upcasts + adds bv).
  - Host-side prep packs every SBUF-resident tensor so it loads in ONE
    DMA (x interleaved [128, 2, HW]; weights pre-tiled/4x-replicated).
  - x DMAs all ride ONE ring (sync) so they transfer serially at full HBM
    bandwidth, part p landing just before projection p needs it; first
    two parts split in half so compute starts ~1us earlier. v'-proj
    evacuates 2 m-blocks per 1-bank PSUM tile to halve rotation stalls.
"""

import numpy as np

import concourse.bacc as bacc
import concourse.bass as bass
import concourse.mybir as mybir
import concourse.tile as tile

B, C, D = 8, 256, 32
HW = 4096                      # 64*64 pixels
NCH = 8                        # n-chunks of 512
CHUNK = 512
NB = 128                       # n-block
MB = 128                       # m-block
NMB = HW // MB                 # 32 m-blocks
VW = 258                       # v' row width: 256 c + colsum-ones + pad
GS = [2] + [3] * 10            # m-blocks per logits group (sum = 32);
                               # small group first so the 2 reserved
                               # flush packs are both 3-wide
GOFF = [0]
for _g in GS:
    GOFF.append(GOFF[-1] + _g)  # group -> first m-block
NG = len(GS)                   # 11 logits groups per chunk
F32 = mybir.dt.float32
BF16 = mybir.dt.bfloat16
FP16 = mybir.dt.float16
AF = mybir.ActivationFunctionType


def build_nc():
    nc = bacc.Bacc("TRN2", target_bir_lowering=False, debug=False)
    t = {}
    t["x"] = nc.dram_tensor("x", [128, 2, HW], FP16, kind="ExternalInput").ap()
    t["wq4"] = nc.dram_tensor("wq4", [128, 2, 128], FP16,
                              kind="ExternalInput").ap()
    t["wk4"] = nc.dram_tensor("wk4", [128, 2, 128], FP16,
                              kind="ExternalInput").ap()
    t["bqk"] = nc.dram_tensor("bqk", [128, 2], F32, kind="ExternalInput").ap()
    t["wvt"] = nc.dram_tensor("wvt", [128, 2, 256], FP16,
                              kind="ExternalInput").ap()
    t["xt"] = nc.dram_tensor("xt", [128, NMB, 256], FP16,
                             kind="ExternalInput").ap()
    t["y"] = nc.dram_tensor("y", [128, NMB, 256], FP16,
                            kind="ExternalOutput").ap()

    with tile.TileContext(nc) as tc:
        _emit(nc, tc, t)
    nc.compile()
    return nc


def _emit(nc, tc, t):
    with (
        tc.tile_pool(name="const", bufs=1) as const,
        tc.tile_pool(name="sb", bufs=1) as sb,
        tc.tile_pool(name="stage", bufs=2) as stage,
    ):
        # ---- constants / weights (one DMA per SBUF tile) -------------
        wq4 = const.tile([128, 2, 128], FP16)  # [c', cc, 4x32 q-weights]
        wk4 = const.tile([128, 2, 128], FP16)
        wvt = const.tile([128, 2, 256], FP16)  # [c', cc, 256 v-weights]
        bqk = const.tile([128, 2], F32)        # col 0: bq4, col 1: bk4
        nc.gpsimd.dma_start(wk4, t["wk4"])
        nc.gpsimd.dma_start(wq4, t["wq4"])
        nc.gpsimd.dma_start(bqk, t["bqk"])

        # ---- persistent SBUF tensors ---------------------------------
        x16 = sb.tile([128, 2, HW], FP16)      # x (fp16): projections
        xt = sb.tile([128, NMB, 256], FP16)    # x transposed [n, c]: residual
        q4 = sb.tile([128, HW], FP16)          # q replicated 4x on partitions
        k4 = sb.tile([128, HW], FP16)
        vp = sb.tile([128, NMB, VW], BF16)     # v' tiles: [m-chunk 128, VW]
        pbuf = [sb.tile([128, 16 * 1024], BF16, tag=f"p{i}", name=f"p{i}")
                for i in range(2)]

        # ones column for the free colsum; pad col zeroed
        nc.gpsimd.memset(vp[:, :, 256:257], 1.0)
        nc.gpsimd.memset(vp[:, :, 257:258], 0.0)

        # x: 8 DMAs of [128, 2, 512], ALL on the sync queue: same-ring DMAs
        # execute serially at full HBM bandwidth, so part p lands at
        # ~8.6+0.9p us, matching the projection cadence (concurrent rings
        # would share bandwidth and deliver part 0 only after ~12.6us)
        xsplits = [0, 256, 512, 768, 1024] + [CHUNK * p for p in range(3, 9)]
        for a, b in zip(xsplits, xsplits[1:]):
            nc.sync.dma_start(x16[:, :, a:b], t["x"][:, :, a:b])
        nc.gpsimd.dma_start(wvt, t["wvt"])
        # residual copy rides the SAME ring as x, serialized behind it so
        # it can't steal bandwidth from the cadenced x parts (first
        # consumer is the first finalize at ~45us)
        nc.sync.dma_start(xt, t["xt"])

        # ---- main loop -----------------------------------------------
        # PSUM: lt 2-bank x3 bufs = 6 banks; "avtr" shared tag (av accum /
        # transpose out / v'-proj / qk-proj) 1 bank x2 bufs = 2. Total 8.
        with tc.tile_pool(name="ps1", bufs=1, space="PSUM") as ps1:

            def qk_proj(w4, bcol, dst, ch, tag):
                s = slice(CHUNK * ch, CHUNK * (ch + 1))
                pt = ps1.tile([128, CHUNK], F32, tag=tag, name="pt", bufs=2)
                for cc in range(2):
                    nc.tensor.matmul(
                        pt, w4[:, cc, :], x16[:, cc, s],
                        start=(cc == 0), stop=(cc == 1),
                    )
                nc.vector.tensor_scalar_add(dst[:, s], pt, bcol)

            def logits_group(ch, g):
                """GS[g] row-packed matmuls (m-blocks GOFF[g]..) + exp.

                Row bands rotate across groups so the next group's
                LDWEIGHTS can pull ahead of in-flight matmuls."""
                sz = GS[g]
                lt = ps1.tile([128, 1536], F32, tag="lt", bufs=2, name="lt")
                ns = slice(CHUNK * ch, CHUNK * (ch + 1))
                for r in range(sz):
                    mb = GOFF[g] + r
                    b = (3 * g + r) % 4
                    nc.tensor.matmul(
                        lt[:, CHUNK * r:CHUNK * (r + 1)],
                        k4[32 * b:32 * (b + 1), MB * mb:MB * (mb + 1)],
                        q4[32 * b:32 * (b + 1), ns],
                        start=True, stop=True, tile_position=(32 * b, 0),
                    )
                dst = pbuf[ch % 2][:, CHUNK * GOFF[g]:CHUNK * GOFF[g + 1]]
                nc.scalar.activation(dst, lt[:, 0:CHUNK * sz], AF.Exp)

            def vt_unit(mb2):
                """v' tiles 2*mb2, 2*mb2+1: 4 matmuls into one 1-bank PSUM
                tile + a single copy (halves the PSUM-rotation stalls)."""
                vt = ps1.tile([128, 2, 256], F32, tag="avtr", bufs=2,
                              name="vt")
                for b in range(2):
                    mb = 2 * mb2 + b
                    ms = slice(MB * mb, MB * (mb + 1))
                    for cc in range(2):
                        nc.tensor.matmul(
                            vt[:, b, :], x16[:, cc, ms], wvt[:, cc, :],
                            start=(cc == 0), stop=(cc == 1),
                        )
                if mb2 % 4 == 0:
                    nc.scalar.activation(vp[:, 2 * mb2:2 * mb2 + 2, 0:256],
                                         vt, AF.Identity)
                else:
                    nc.vector.tensor_copy(vp[:, 2 * mb2:2 * mb2 + 2, 0:256],
                                          vt)

            def av_unit(ch, j, mc):
                off = CHUNK * mc  # m-blocks pack contiguously in pbuf
                nc.tensor.matmul(
                    t["avps"], pbuf[ch % 2][:, off + NB * j:off + NB * (j + 1)],
                    vp[:, mc, :],
                    start=(mc == 0), stop=(mc == 31),
                )

            def finalize(ch, j, ysb):
                """All-DVE: normalize by 1/colsum, add the [n, c]-layout
                residual; output stays [n, c] (host transposes back)."""
                avps = t["avps"]
                recip = stage.tile([128, 1], F32, tag="recip", name="recip")
                nc.vector.reciprocal(recip, avps[:, 256:257])
                normt = stage.tile([128, 256], FP16, tag="normt", name="normt")
                nc.vector.tensor_scalar_mul(normt, avps[:, 0:256], recip)
                nc.vector.tensor_tensor(
                    out=ysb[:, j, :], in0=normt, in1=xt[:, 4 * ch + j, :],
                    op=mybir.AluOpType.add,
                )

            # k4 chunks required before logits pack g can run (cols 384g..)
            KREQ = [min((128 * GOFF[g + 1] - 1) // CHUNK, NCH - 1)
                    for g in range(NG)]
            for ch in range(NCH + 1):
                # filler units for this pipeline stage:
                #  ch == 0  -> k4/q4 proj (u<8), 16 paired v'-proj units
                #  ch >= 1  -> 128 AV matmuls of chunk ch-1 (+finalize/4)
                n_units = (NCH + NMB // 2) if ch == 0 else 128
                ysb = None
                if ch > 0:
                    ysb = stage.tile([128, 4, 256], FP16, tag="y", name="ysb")
                g_next = 0
                # packs are emitted in back-to-back pairs so the 2nd pack's
                # LDWEIGHTS hide under the 1st pack's MMs
                for u in range(n_units):
                    if ch < NCH:
                        while (g_next < NG
                               and g_next <= (u * NG) // n_units
                               and not (ch == 0 and KREQ[g_next] >= u)):
                            logits_group(ch, g_next)
                            g_next += 1
                            if ch >= 1 and g_next < NG:
                                logits_group(ch, g_next)
                                g_next += 1
                    if ch == 0:
                        if u < NCH:
                            qk_proj(wk4, bqk[:, 1:2], k4, u, "avtr")
                            qk_proj(wq4, bqk[:, 0:1], q4, u, "avtr")
                        else:
                            vt_unit(u - NCH)
                    else:
                        j, mc = divmod(u, 32)
                        if mc == 0:
                            t["avps"] = ps1.tile([128, VW], F32, tag="avtr",
                                                 bufs=2, name="avps")
                        av_unit(ch - 1, j, mc)
                        if mc == 31:
                            finalize(ch - 1, j, ysb)
                if ch < NCH:
                    while g_next < NG:
                        logits_group(ch, g_next)
                        g_next += 1
                if ch > 0:
                    nc.sync.dma_start(t["y"][:, 4 * (ch - 1):4 * ch, :], ysb)


# ---------------------------------------------------------------------
# host-side wrapper
# ---------------------------------------------------------------------
_CACHE = {}


def _prep_shared(Wq, bq, Wk, bk, Wv, bv):
    wq4 = np.tile(Wq.T, (1, 4)).reshape(2, 128, 128).transpose(1, 0, 2)
    wk4 = np.tile(Wk.T, (1, 4)).reshape(2, 128, 128).transpose(1, 0, 2)
    bqk = np.stack([np.tile(bq, 4), np.tile(bk, 4)], axis=1)
    wvt = Wv.T.reshape(2, 128, 256).transpose(1, 0, 2)
    return {"wq4": np.ascontiguousarray(wq4, dtype=np.float16),
            "wk4": np.ascontiguousarray(wk4, dtype=np.float16),
            "bqk": np.ascontiguousarray(bqk, dtype=np.float32),
            "wvt": np.ascontiguousarray(wvt, dtype=np.float16)}


def make_in_maps(x, Wq, bq, Wk, bk, Wv, bv):
    x = np.asarray(x, dtype=np.float32).reshape(B, C, HW)
    xi = x.reshape(B, 2, 128, HW).transpose(0, 2, 1, 3).astype(np.float16)
    # transposed residual copy: xt[p, nb, c] = x[c, 128*nb + p]
    xts = x.reshape(B, C, NMB, 128).transpose(0, 3, 2, 1).astype(np.float16)
    shared = _prep_shared(*(np.asarray(a, dtype=np.float32)
                            for a in (Wq, bq, Wk, bk, Wv, bv)))
    return [{"x": np.ascontiguousarray(xi[b]),
             "xt": np.ascontiguousarray(xts[b]), **shared} for b in range(B)]


def postprocess(res, bv):
    yt = np.stack([res.results[b]["y"] for b in range(B)])  # [B,128,NMB,256]
    y = yt.astype(np.float32).transpose(0, 3, 2, 1)  # [B, c, nb, p]
    y = np.ascontiguousarray(y).reshape(B, C, HW)
    y += np.asarray(bv, dtype=np.float32)[None, :, None]
    return y.reshape(B, C, 64, 64)


def kernel(x, Wq, bq, Wk, bk, Wv, bv):
    from concourse.bass_utils import run_bass_kernel_spmd

    in_maps = make_in_maps(x, Wq, bq, Wk, bk, Wv, bv)
    if "nc" not in _CACHE:
        _CACHE["nc"] = build_nc()
    res = run_bass_kernel_spmd(_CACHE["nc"], in_maps, core_ids=list(range(B)))
    return postprocess(res, bv)
